# revision 9
# baseline (speedup 1.0000x reference)
"""Trainium2 Bass kernel for nn_ContinualForecaster (scatter_memory).

Strategy: data-parallel over batch (B=8 -> 8 NeuronCores, one batch element
per core). The T=256 sequential state recurrence on M,S [128,128] is
reformulated as a chunked parallel scan (2 chunks of L=128):

  err_t = M k_t - v_t ; S = et*S - th*err_t k_t^T ; M = (1-al)*M + S

is linear in (M, S) given the matvecs z_t = M_{t-1} k_t. Within a chunk the
unknown update vectors w_t = -th_t(z_t - v_t) satisfy a strictly-triangular
linear system W = (I-N)^{-1} R whose coefficients come from cumulative
products of (et, 1-al) (computed stably in log space) and the key Gram matrix
K K^T. (I-N)^{-1} is formed with Neumann doubling (N is nilpotent), all as
128x128 TensorEngine matmuls. Only the final M is needed downstream (the
reference consumes fused[:, -1, :] only), so per chunk we emit closed-form
state updates M_L, S_L via two more matmuls.
"""

import numpy as np
from contextlib import ExitStack

import sys

for _p in ("/opt/trn_rl_repo",):
    if _p not in sys.path:
        sys.path.append(_p)

B, T, DI, D = 8, 256, 64, 128
PRED_LEN, OUT_DIM = 96, 7
OUTN = PRED_LEN * OUT_DIM  # 672
L = 128
NCHUNK = T // L
LN_EPS = 1e-5

_CACHE = {}


def _build():
    import concourse.bass as bass
    import concourse.tile as tile
    from concourse import bacc, mybir

    f32 = mybir.dt.float32
    AF = mybir.ActivationFunctionType
    OP = mybir.AluOpType

    nc = bacc.Bacc()

    def din(name, shape):
        return nc.declare_dram_parameter(name, shape, f32, isOutput=False)

    xT_d = din("xT", [DI, T])
    Wb_d = din("Wb", [DI, D])
    bb_d = din("bb", [D, 1])
    Wk_d = din("Wk", [D, D])
    Wv_d = din("Wv", [D, D])
    Wq_d = din("Wq", [D, D])
    Wm0_d = din("Wm0", [D, 1])
    Wm1_d = din("Wm1", [D, 1])
    Wm2_d = din("Wm2", [D, 1])
    Wf1_d = din("Wf1", [D, D])
    Wf2_d = din("Wf2", [D, D])
    bf_d = din("bf", [D, 1])
    W1_d = din("W1", [D, D])
    b1_d = din("b1", [D, 1])
    g1_d = din("g1", [1, D])
    be1_d = din("be1", [1, D])
    W2_d = din("W2", [D, OUTN])
    b2_d = din("b2", [1, OUTN])
    mlti_d = din("mlti", [D, D])  # 1.0 where part >= free (lower incl diag)
    muti_d = din("muti", [D, D])  # 1.0 where free >= part (upper incl diag)
    iden_d = din("iden", [D, D])
    out_d = nc.declare_dram_parameter("out", [1, OUTN], f32, isOutput=True)

    with ExitStack() as ctx:
        tc = ctx.enter_context(tile.TileContext(nc))
        cst = ctx.enter_context(tc.tile_pool(name="cst", bufs=1))
        wrk = ctx.enter_context(tc.tile_pool(name="wrk", bufs=2))
        tny = ctx.enter_context(tc.tile_pool(name="tny", bufs=2))
        psA = ctx.enter_context(
            tc.tile_pool(name="psA", bufs=2, space=bass.MemorySpace.PSUM)
        )
        psB = ctx.enter_context(
            tc.tile_pool(name="psB", bufs=4, space=bass.MemorySpace.PSUM)
        )
        psT = ctx.enter_context(
            tc.tile_pool(name="psT", bufs=2, space=bass.MemorySpace.PSUM)
        )

        # ---- load constants to SBUF ----
        def load(dram, shape, tag):
            t = cst.tile(shape, f32, tag=tag)
            nc.gpsimd.dma_start(t[:], dram[:])
            return t

        xT = load(xT_d, [DI, T], "xT")
        Wb = load(Wb_d, [DI, D], "Wb")
        bb = load(bb_d, [D, 1], "bb")
        Wk = load(Wk_d, [D, D], "Wk")
        Wv = load(Wv_d, [D, D], "Wv")
        Wq = load(Wq_d, [D, D], "Wq")
        Wm0 = load(Wm0_d, [D, 1], "Wm0")
        Wm1 = load(Wm1_d, [D, 1], "Wm1")
        Wm2 = load(Wm2_d, [D, 1], "Wm2")
        Wf1 = load(Wf1_d, [D, D], "Wf1")
        Wf2 = load(Wf2_d, [D, D], "Wf2")
        bf = load(bf_d, [D, 1], "bf")
        W1 = load(W1_d, [D, D], "W1")
        b1 = load(b1_d, [D, 1], "b1")
        g1 = load(g1_d, [1, D], "g1")
        be1 = load(be1_d, [1, D], "be1")
        W2 = load(W2_d, [D, OUTN], "W2")
        b2 = load(b2_d, [1, OUTN], "b2")
        mlti = load(mlti_d, [D, D], "mlti")
        muti = load(muti_d, [D, D], "muti")
        iden = load(iden_d, [D, D], "iden")

        ones_col = cst.tile([1, D], f32, tag="ones_col")
        nc.vector.memset(ones_col[:], 1.0)
        one11 = cst.tile([1, 1], f32, tag="one11")
        nc.vector.memset(one11[:], 1.0)
        ones_row = cst.tile([1, T], f32, tag="ones_row")
        nc.vector.memset(ones_row[:], 1.0)

        ncopy = [0]

        def p2s(psum_ap, shape, tag, pool=wrk):
            """PSUM -> SBUF copy, alternating DVE/ACT to balance engines."""
            t = pool.tile(shape, f32, tag=tag)
            if ncopy[0] % 2 == 0:
                nc.vector.tensor_copy(t[:], psum_ap)
            else:
                nc.scalar.copy(t[:], psum_ap)
            ncopy[0] += 1
            return t

        def row_to_col(row_ap, n, tag):
            p = psT.tile([n, 1], f32, tag="tp")
            nc.tensor.matmul(p[:], row_ap, one11[:], start=True, stop=True)
            return p2s(p[:], [n, 1], tag, pool=tny)

        def bcast(row_ap, n, tag):
            """[1,n] row -> [128,n] PSUM broadcast."""
            p = psB.tile([D, n], f32, tag="mm")
            nc.tensor.matmul(p[:], ones_col[:], row_ap, start=True, stop=True)
            return p

        # ---- stage 1: features fT = gelu(Wb^T xT + bb) [D, T] ----
        pf = psA.tile([D, T], f32, tag="big")
        nc.tensor.matmul(pf[:], Wb[:], xT[:], start=True, stop=True)
        fT = cst.tile([D, T], f32, tag="fT")
        nc.scalar.activation(fT[:], pf[:], AF.Gelu_apprx_tanh, bias=bb[:])

        # ---- stage 2: projections ----
        pkT = psA.tile([D, T], f32, tag="big")
        nc.tensor.matmul(pkT[:], Wk[:], fT[:], start=True, stop=True)
        kT = p2s(pkT[:], [D, T], "kT", pool=cst)

        Kc = []
        Vc = []
        for c in range(NCHUNK):
            pk = psB.tile([L, D], f32, tag="mm")
            nc.tensor.matmul(
                pk[:], fT[:, c * L : (c + 1) * L], Wk[:], start=True, stop=True
            )
            Kc.append(p2s(pk[:], [L, D], f"Kc{c}", pool=cst))
            pv = psB.tile([L, D], f32, tag="mm")
            nc.tensor.matmul(
                pv[:], fT[:, c * L : (c + 1) * L], Wv[:], start=True, stop=True
            )
            Vc.append(p2s(pv[:], [L, D], f"Vc{c}", pool=cst))

        # ---- stage 3: meta scalars (rows [1, T]) ----
        pm0 = psT.tile([1, T], f32, tag="tp")
        nc.tensor.matmul(pm0[:], Wm0[:], fT[:], start=True, stop=True)
        th_row = cst.tile([1, T], f32, tag="th_row")
        nc.scalar.activation(th_row[:], pm0[:], AF.Sigmoid)
        nc.scalar.mul(th_row[:], th_row[:], 0.01)

        pm1 = psT.tile([1, T], f32, tag="tp")
        nc.tensor.matmul(pm1[:], Wm1[:], fT[:], start=True, stop=True)
        et_row = tny.tile([1, T], f32, tag="et_row")
        nc.scalar.activation(et_row[:], pm1[:], AF.Sigmoid)
        log_et = tny.tile([1, T], f32, tag="log_et")
        nc.scalar.activation(log_et[:], et_row[:], AF.Ln)

        pm2 = psT.tile([1, T], f32, tag="tp")
        nc.tensor.matmul(pm2[:], Wm2[:], fT[:], start=True, stop=True)
        p_row = tny.tile([1, T], f32, tag="p_row")
        nc.scalar.activation(p_row[:], pm2[:], AF.Sigmoid)
        nc.vector.tensor_scalar(p_row[:], p_row[:], -0.1, 1.0, OP.mult, OP.add)
        log_p = tny.tile([1, T], f32, tag="log_p")
        nc.scalar.activation(log_p[:], p_row[:], AF.Ln)

        # prefix sums (inclusive) with a leading zero -> [1, T+1]
        le_ext = cst.tile([1, T + 1], f32, tag="le_ext")
        nc.vector.memset(le_ext[:, 0:1], 0.0)
        nc.vector.tensor_tensor_scan(
            le_ext[:, 1 : T + 1], ones_row[:], log_et[:], 0.0, OP.mult, OP.add
        )
        la_ext = cst.tile([1, T + 1], f32, tag="la_ext")
        nc.vector.memset(la_ext[:, 0:1], 0.0)
        nc.vector.tensor_tensor_scan(
            la_ext[:, 1 : T + 1], ones_row[:], log_p[:], 0.0, OP.mult, OP.add
        )

        # ---- chunks ----
        MT_sb = None
        ST_sb = None
        for c in range(NCHUNK):
            t0 = c * L
            last = c == NCHUNK - 1
            le_seg = le_ext[:, t0 + 1 : t0 + L + 1]
            la_seg = la_ext[:, t0 + 1 : t0 + L + 1]

            le_col = row_to_col(le_seg, L, f"le_col{c}")
            la_col = row_to_col(la_seg, L, f"la_col{c}")
            th_col = row_to_col(th_row[:, t0 : t0 + L], L, f"th_col{c}")

            # tables: Ftil^T = exp(-max(le_row-le_col,0)) * lower_mask
            #         Gtil   = exp( min(la_row-la_col,0)) * upper_mask
            le_b = bcast(le_seg, L, f"le_b{c}")
            dpos = wrk.tile([L, L], f32, tag="dpos")
            nc.vector.tensor_scalar(
                dpos[:], le_b[:], le_col[:], 0.0, OP.subtract, OP.max
            )
            FtT = wrk.tile([L, L], f32, tag="FtT")
            nc.scalar.activation(FtT[:], dpos[:], AF.Exp, scale=-1.0)
            nc.vector.tensor_mul(FtT[:], FtT[:], mlti[:])

            la_b = bcast(la_seg, L, f"la_b{c}")
            dneg = wrk.tile([L, L], f32, tag="dneg")
            nc.vector.tensor_scalar(
                dneg[:], la_b[:], la_col[:], 0.0, OP.subtract, OP.min
            )
            Gt = wrk.tile([L, L], f32, tag="Gt")
            nc.scalar.activation(Gt[:], dneg[:], AF.Exp)
            nc.vector.tensor_mul(Gt[:], Gt[:], muti[:])

            pC = psB.tile([L, L], f32, tag="mm")
            nc.tensor.matmul(pC[:], FtT[:], Gt[:], start=True, stop=True)

            # Gram matrix Psi = K K^T
            pPsi = psB.tile([L, L], f32, tag="mm")
            nc.tensor.matmul(
                pPsi[:],
                kT[:, t0 : t0 + L],
                kT[:, t0 : t0 + L],
                start=True,
                stop=True,
            )

            # C shifted right in free dim; col 0 zero. C[j,tau]=0 for tau<j
            # already, so C_sh is strictly-upper by construction.
            C_sh = wrk.tile([L, L], f32, tag="C_sh")
            nc.vector.memset(C_sh[:, 0:1], 0.0)
            nc.vector.tensor_copy(C_sh[:, 1:L], pC[:, 0 : L - 1])
            cL_col = p2s(pC[:, L - 1 : L], [L, 1], f"cL{c}", pool=tny)

            # NT[j,t] = -th_t * C[j,t-1] * Psi[j,t]  (strictly upper)
            NT_a = wrk.tile([L, L], f32, tag="NT_a")
            nc.vector.tensor_mul(NT_a[:], C_sh[:], pPsi[:])
            th_b = bcast(th_row[:, t0 : t0 + L], L, f"th_b{c}")
            NT = wrk.tile([L, L], f32, tag="NT")
            nc.vector.scalar_tensor_tensor(
                NT[:], th_b[:], -1.0, NT_a[:], OP.mult, OP.mult
            )

            # N = NT^T via PE transpose
            pN = psB.tile([L, L], f32, tag="mm")
            nc.tensor.transpose(pN[:], NT[:], iden[:])
            X = p2s(pN[:], [L, L], "Xk", pool=wrk)
            Y = NT

            # INVT = (I - NT)^{-1} built by Neumann doubling (transposed so
            # W = INV @ R becomes matmul(lhsT=INVT, rhs=R)).
            INVT = wrk.tile([L, L], f32, tag="INVT")
            nc.vector.tensor_add(INVT[:], NT[:], iden[:])
            for lev in range(1, 7):
                pX2 = psA.tile([L, L], f32, tag="big")
                nc.tensor.matmul(pX2[:], Y[:], X[:], start=True, stop=True)
                X2 = p2s(pX2[:], [L, L], "Xk", pool=wrk)
                if lev < 6:
                    pY2 = psA.tile([L, L], f32, tag="big")
                    nc.tensor.matmul(pY2[:], X[:], Y[:], start=True, stop=True)
                    Y = p2s(pY2[:], [L, L], "Yk", pool=wrk)
                X = X2
                pIU = psA.tile([L, L], f32, tag="big")
                nc.tensor.matmul(pIU[:], X[:], INVT[:], start=True, stop=True)
                INVT2 = wrk.tile([L, L], f32, tag="INVT")
                nc.vector.tensor_add(INVT2[:], INVT[:], pIU[:])
                INVT = INVT2

            # R
            if c == 0:
                R = wrk.tile([L, D], f32, tag="R")
                nc.vector.tensor_scalar(R[:], Vc[c][:], th_col[:], None, OP.mult)
            else:
                la_prev_col = row_to_col(
                    la_ext[:, t0 : t0 + L], L, f"la_prev{c}"
                )
                # neg la0 / le0 broadcast columns for exp biases
                nla0 = psT.tile([D, 1], f32, tag="tp")
                nc.tensor.matmul(
                    nla0[:], ones_col[:], la_ext[:, t0 : t0 + 1], start=True, stop=True
                )
                nla0_sb = tny.tile([D, 1], f32, tag="nla0_sb")
                nc.scalar.mul(nla0_sb[:], nla0[:], -1.0)
                A_prev = tny.tile([L, 1], f32, tag="A_prev")
                nc.scalar.activation(
                    A_prev[:], la_prev_col[:], AF.Exp, bias=nla0_sb[:]
                )

                nle0 = psT.tile([D, 1], f32, tag="tp")
                nc.tensor.matmul(
                    nle0[:], ones_col[:], le_ext[:, t0 : t0 + 1], start=True, stop=True
                )
                nle0_sb = tny.tile([D, 1], f32, tag="nle0_sb")
                nc.scalar.mul(nle0_sb[:], nle0[:], -1.0)
                E_col = tny.tile([L, 1], f32, tag="E_col")
                nc.scalar.activation(E_col[:], le_col[:], AF.Exp, bias=nle0_sb[:])

                # b row = E_col^T @ Gtil ; b_prev = shifted
                pb = psT.tile([1, L], f32, tag="tp")
                nc.tensor.matmul(pb[:], E_col[:], Gt[:], start=True, stop=True)
                b_row = p2s(pb[:], [1, L], "b_row", pool=tny)
                b_sh = tny.tile([1, L], f32, tag="b_sh")
                nc.vector.memset(b_sh[:, 0:1], 0.0)
                nc.vector.tensor_copy(b_sh[:, 1:L], b_row[:, 0 : L - 1])
                b_prev = row_to_col(b_sh[:], L, f"b_prev{c}")

                # A_L, b_L broadcast columns (scalars of this chunk)
                dl = tny.tile([1, 1], f32, tag="dl")
                nc.vector.tensor_scalar(
                    dl[:],
                    la_ext[:, t0 + L : t0 + L + 1],
                    la_ext[:, t0 : t0 + 1],
                    None,
                    OP.subtract,
                )
                nc.scalar.activation(dl[:], dl[:], AF.Exp)
                pAL = psT.tile([D, 1], f32, tag="tp")
                nc.tensor.matmul(pAL[:], ones_col[:], dl[:], start=True, stop=True)
                AL_col = p2s(pAL[:], [D, 1], "AL_col", pool=tny)
                pbL = psT.tile([D, 1], f32, tag="tp")
                nc.tensor.matmul(
                    pbL[:], ones_col[:], b_row[:, L - 1 : L], start=True, stop=True
                )
                bL_col = p2s(pbL[:], [D, 1], "bL_col", pool=tny)

                pZM = psA.tile([L, D], f32, tag="big")
                nc.tensor.matmul(
                    pZM[:], kT[:, t0 : t0 + L], MT_sb[:], start=True, stop=True
                )
                pZS = psA.tile([L, D], f32, tag="big")
                nc.tensor.matmul(
                    pZS[:], kT[:, t0 : t0 + L], ST_sb[:], start=True, stop=True
                )
                t1 = wrk.tile([L, D], f32, tag="t1")
                nc.vector.tensor_scalar(t1[:], pZM[:], A_prev[:], None, OP.mult)
                t2 = wrk.tile([L, D], f32, tag="t2")
                nc.vector.scalar_tensor_tensor(
                    t2[:], pZS[:], b_prev[:], t1[:], OP.mult, OP.add
                )
                nc.vector.tensor_sub(t2[:], t2[:], Vc[c][:])
                R = wrk.tile([L, D], f32, tag="R")
                nc.vector.tensor_scalar(
                    R[:], t2[:], th_col[:], -1.0, OP.mult, OP.mult
                )

            # W = INV @ R
            pW = psA.tile([L, D], f32, tag="big")
            nc.tensor.matmul(pW[:], INVT[:], R[:], start=True, stop=True)
            W = p2s(pW[:], [L, D], "W", pool=wrk)

            # state update
            Wp = wrk.tile([L, D], f32, tag="Wp")
            nc.vector.tensor_scalar(Wp[:], W[:], cL_col[:], None, OP.mult)
            pMTc = psA.tile([D, D], f32, tag="big")
            nc.tensor.matmul(pMTc[:], Kc[c][:], Wp[:], start=True, stop=True)

            if c == 0:
                MT_sb = p2s(pMTc[:], [D, D], "MT", pool=cst)
                # S update needed only when a later chunk consumes it
                leL_b = psT.tile([D, 1], f32, tag="tp")
                nc.tensor.matmul(
                    leL_b[:],
                    ones_col[:],
                    le_ext[:, t0 + L : t0 + L + 1],
                    start=True,
                    stop=True,
                )
                leL_sb = p2s(leL_b[:], [D, 1], "leL_sb", pool=tny)
                FL_col = tny.tile([L, 1], f32, tag="FL_col")
                nc.scalar.activation(
                    FL_col[:], le_col[:], AF.Exp, scale=-1.0, bias=leL_sb[:]
                )
                Wpp = wrk.tile([L, D], f32, tag="Wpp")
                nc.vector.tensor_scalar(Wpp[:], W[:], FL_col[:], None, OP.mult)
                pSTc = psA.tile([D, D], f32, tag="big")
                nc.tensor.matmul(pSTc[:], Kc[c][:], Wpp[:], start=True, stop=True)
                ST_sb = p2s(pSTc[:], [D, D], "ST", pool=cst)
            else:
                a1 = wrk.tile([D, D], f32, tag="a1")
                nc.vector.scalar_tensor_tensor(
                    a1[:], MT_sb[:], AL_col[:], pMTc[:], OP.mult, OP.add
                )
                MT2 = wrk.tile([D, D], f32, tag="MT2")
                nc.vector.scalar_tensor_tensor(
                    MT2[:], ST_sb[:], bL_col[:], a1[:], OP.mult, OP.add
                )
                MT_sb = MT2

        # ---- head (last timestep only) ----
        f_last = fT[:, T - 1 : T]
        pq = psT.tile([D, 1], f32, tag="tp")
        nc.tensor.matmul(pq[:], Wq[:], f_last, start=True, stop=True)
        q_col = p2s(pq[:], [D, 1], "q_col", pool=tny)

        pmm = psT.tile([D, 1], f32, tag="tp")
        nc.tensor.matmul(pmm[:], MT_sb[:], q_col[:], start=True, stop=True)
        m_col = p2s(pmm[:], [D, 1], "m_col", pool=tny)

        pg = psT.tile([D, 1], f32, tag="tp")
        nc.tensor.matmul(pg[:], Wf1[:], f_last, start=True, stop=False)
        nc.tensor.matmul(pg[:], Wf2[:], m_col[:], start=False, stop=True)
        gate = tny.tile([D, 1], f32, tag="gate")
        nc.scalar.activation(gate[:], pg[:], AF.Sigmoid, bias=bf[:])

        dfm = tny.tile([D, 1], f32, tag="dfm")
        nc.vector.tensor_sub(dfm[:], f_last, m_col[:])
        fused = tny.tile([D, 1], f32, tag="fused")
        nc.vector.scalar_tensor_tensor(
            fused[:], dfm[:], gate[:], m_col[:], OP.mult, OP.add
        )

        py = psT.tile([D, 1], f32, tag="tp")
        nc.tensor.matmul(py[:], W1[:], fused[:], start=True, stop=True)
        y_col = tny.tile([D, 1], f32, tag="y_col")
        nc.scalar.activation(y_col[:], py[:], AF.Identity, bias=b1[:])

        pyr = psT.tile([1, D], f32, tag="tp")
        nc.tensor.matmul(pyr[:], y_col[:], iden[:], start=True, stop=True)
        y_row = tny.tile([1, D], f32, tag="y_row")
        nc.vector.tensor_copy(y_row[:], pyr[:])

        mu = tny.tile([1, 1], f32, tag="mu")
        nc.vector.tensor_reduce(mu[:], y_row[:], mybir.AxisListType.X, OP.add)
        nc.scalar.mul(mu[:], mu[:], 1.0 / D)
        xc = tny.tile([1, D], f32, tag="xc")
        nc.vector.tensor_scalar(xc[:], y_row[:], mu[:], None, OP.subtract)
        sq = tny.tile([1, D], f32, tag="sq")
        nc.vector.tensor_mul(sq[:], xc[:], xc[:])
        var = tny.tile([1, 1], f32, tag="var")
        nc.vector.tensor_reduce(var[:], sq[:], mybir.AxisListType.X, OP.add)
        eps_t = tny.tile([1, 1], f32, tag="eps_t")
        nc.vector.memset(eps_t[:], LN_EPS)
        sd = tny.tile([1, 1], f32, tag="sd")
        nc.scalar.activation(sd[:], var[:], AF.Sqrt, scale=1.0 / D, bias=eps_t[:])
        rstd = tny.tile([1, 1], f32, tag="rstd")
        nc.vector.reciprocal(rstd[:], sd[:])

        hh = tny.tile([1, D], f32, tag="hh")
        nc.vector.tensor_scalar(hh[:], xc[:], rstd[:], None, OP.mult)
        nc.vector.tensor_mul(hh[:], hh[:], g1[:])
        nc.vector.tensor_add(hh[:], hh[:], be1[:])
        h_row = tny.tile([1, D], f32, tag="h_row")
        nc.scalar.activation(h_row[:], hh[:], AF.Gelu_apprx_tanh)

        ph = psT.tile([D, 1], f32, tag="tp")
        nc.tensor.matmul(ph[:], h_row[:], one11[:], start=True, stop=True)
        h_col = p2s(ph[:], [D, 1], "h_col", pool=tny)

        po1 = psT.tile([1, 512], f32, tag="tp")
        nc.tensor.matmul(po1[:], h_col[:], W2[:, 0:512], start=True, stop=True)
        po2 = psT.tile([1, OUTN - 512], f32, tag="tp")
        nc.tensor.matmul(po2[:], h_col[:], W2[:, 512:OUTN], start=True, stop=True)
        orow = tny.tile([1, OUTN], f32, tag="orow")
        nc.vector.tensor_add(orow[:, 0:512], po1[:], b2[:, 0:512])
        nc.vector.tensor_add(orow[:, 512:OUTN], po2[:], b2[:, 512:OUTN])

        nc.gpsimd.dma_start(out_d[:], orow[:])

    nc.finalize()
    return nc


def _prep_maps(inputs):
    f = np.float32
    x = np.asarray(inputs["x"], f)
    idx = np.arange(D)
    mlti = (idx[:, None] >= idx[None, :]).astype(f)
    muti = (idx[None, :] >= idx[:, None]).astype(f)
    iden = np.eye(D, dtype=f)
    base = {
        "Wb": np.ascontiguousarray(np.asarray(inputs["W_b"], f)),
        "bb": np.asarray(inputs["b_b"], f).reshape(D, 1).copy(),
        "Wk": np.ascontiguousarray(np.asarray(inputs["Wk"], f)),
        "Wv": np.ascontiguousarray(np.asarray(inputs["Wv"], f)),
        "Wq": np.ascontiguousarray(np.asarray(inputs["Wq"], f)),
        "Wm0": np.asarray(inputs["W_m"], f)[:, 0:1].copy(),
        "Wm1": np.asarray(inputs["W_m"], f)[:, 1:2].copy(),
        "Wm2": np.asarray(inputs["W_m"], f)[:, 2:3].copy(),
        "Wf1": np.ascontiguousarray(np.asarray(inputs["W_f"], f)[:D]),
        "Wf2": np.ascontiguousarray(np.asarray(inputs["W_f"], f)[D:]),
        "bf": np.asarray(inputs["b_f"], f).reshape(D, 1).copy(),
        "W1": np.ascontiguousarray(np.asarray(inputs["W1"], f)),
        "b1": np.asarray(inputs["b1"], f).reshape(D, 1).copy(),
        "g1": np.asarray(inputs["g1"], f).reshape(1, D).copy(),
        "be1": np.asarray(inputs["be1"], f).reshape(1, D).copy(),
        "W2": np.ascontiguousarray(np.asarray(inputs["W2"], f)),
        "b2": np.asarray(inputs["b2"], f).reshape(1, OUTN).copy(),
        "mlti": mlti,
        "muti": muti,
        "iden": iden,
    }
    maps = []
    for b in range(B):
        m = dict(base)
        m["xT"] = np.ascontiguousarray(x[b].T)
        maps.append(m)
    return maps


def kernel(**inputs):
    from concourse.bass_utils import run_bass_kernel_spmd

    if "nc" not in _CACHE:
        _CACHE["nc"] = _build()
    nc = _CACHE["nc"]
    maps = _prep_maps(inputs)
    res = run_bass_kernel_spmd(nc, maps, core_ids=list(range(B)))
    outs = [res.results[i]["out"].reshape(PRED_LEN, OUT_DIM) for i in range(B)]
    return np.stack(outs).astype(np.float32)
